# revision 11
# baseline (speedup 1.0000x reference)
"""Pairwise module kernel for Trainium2 (8 NeuronCores, SPMD).

Computes out[b, i, j, p] = (h[b,i,:] @ W1.T)[p] + (h[b,j,:] @ W2.T)[p] + bias[p]
where W1 = W[:, :D], W2 = W[:, D:].

Shapes (hardcoded): h (2, 512, 256) f32, W (128, 512) f32, b (128,) f32.
Output: (2, 512, 512, 128) f32 = 256 MB.

Sharding: the (B*L = 1024) "query rows" i are split into 8 contiguous blocks
of 128; core c handles batch b = c // 4, rows i in [(c%4)*128, (c%4)*128+128).

Per-core device program:
  - load hT (d-major h[b].T), hrT (this core's 128 rows, d-major), WT, bias
  - PE: p2T[p, j]  = W2 @ h[b].T          (2 accumulating matmuls, K=128 each)
        p1bT[p, i] = W1 @ h_rows.T + bias (2 matmuls + rank-1 bias matmul)
  - DVE: for each of the 128 rows i: one tensor_scalar add
        out_tile[p, j] = p2T[p, j] + p1bT[p, i]   (per-partition scalar bcast)
  - DMA: store (p, i, j)-layout shard to DRAM in 2 MB contiguous chunks.

The per-core DRAM shard is (p, i_local, j); the host transposes/reassembles
into the (b, i, j, p) full output (host-side layout work only, no arithmetic).
"""

import numpy as np

B, L, D, P = 2, 512, 256, 128
NCORES = 8
BLOCKS_PER_B = 4  # 4 i-blocks of 128 per batch element
G = 8  # i-rows per output SBUF tile / per store DMA (G*512*128*4 = 2 MB)
F32 = None  # set in _build (mybir import kept inside functions)


def _build_program(act_rows=0):
    """Build the SPMD Bass program (identical on all 8 cores).

    act_rows: how many of the 128 broadcast-add rows to route to the scalar
    (ACT) engine instead of the vector engine (DVE) — load-balancing knob.
    """
    import concourse.bacc as bacc
    import concourse.mybir as mybir
    from concourse.tile import TileContext

    f32 = mybir.dt.float32
    nc = bacc.Bacc("TRN2", target_bir_lowering=False, debug=False, enable_partition_id=False)

    hT_d = nc.dram_tensor("hT", [D, L], f32, kind="ExternalInput")  # h[b].T
    hrT_d = nc.dram_tensor("hrT", [D, P], f32, kind="ExternalInput")  # rows.T
    WT_d = nc.dram_tensor("WT", [2 * D, P], f32, kind="ExternalInput")  # W.T
    bias_d = nc.dram_tensor("bias", [1, P], f32, kind="ExternalInput")
    out_d = nc.dram_tensor("out", [P, P * L], f32, kind="ExternalOutput")

    n_groups = P // G

    with TileContext(nc) as tc:
        with (
            tc.tile_pool(name="const", bufs=1) as cpool,
            tc.tile_pool(name="outp", bufs=4) as opool,
            tc.tile_pool(name="psum", bufs=2, space="PSUM") as ppool,
        ):
            # ---- load inputs (all tiny: ~1 MB total) ----
            # p2T's deps (hT, w2t) go first, split across both HWDGE rings,
            # so the PE -> DVE -> store pipeline lights up as early as
            # possible; p1bT's deps follow.
            hT0 = cpool.tile([P, L], f32, tag="hT0")
            hT1 = cpool.tile([P, L], f32, tag="hT1")
            w1t0 = cpool.tile([P, P], f32, tag="w1t0")
            w1t1 = cpool.tile([P, P], f32, tag="w1t1")
            w2t0 = cpool.tile([P, P], f32, tag="w2t0")
            w2t1 = cpool.tile([P, P], f32, tag="w2t1")
            hrT0 = cpool.tile([P, P], f32, tag="hrT0")
            hrT1 = cpool.tile([P, P], f32, tag="hrT1")
            bias_sb = cpool.tile([1, P], f32, tag="bias")

            nc.sync.dma_start(out=hT0[:], in_=hT_d[0:P, :])
            nc.scalar.dma_start(out=w2t0[:], in_=WT_d[2 * P : 3 * P, :])
            nc.scalar.dma_start(out=w2t1[:], in_=WT_d[3 * P : 4 * P, :])
            nc.sync.dma_start(out=hT1[:], in_=hT_d[P : 2 * P, :])
            nc.scalar.dma_start(out=hrT0[:], in_=hrT_d[0:P, :])
            nc.sync.dma_start(out=w1t0[:], in_=WT_d[0:P, :])
            nc.scalar.dma_start(out=hrT1[:], in_=hrT_d[P : 2 * P, :])
            nc.sync.dma_start(out=w1t1[:], in_=WT_d[P : 2 * P, :])
            nc.scalar.dma_start(out=bias_sb[:], in_=bias_d[:, :])

            ones_sb = cpool.tile([1, P], f32, tag="ones")
            nc.vector.memset(ones_sb[:], 1.0)

            # ---- PE: p2T[p, j] = W2 @ hT  (out (128, 512), K = 256 in 2) ----
            p2T_ps = ppool.tile([P, L], f32, tag="p2ps")
            nc.tensor.matmul(p2T_ps[:], w2t0[:], hT0[:], start=True, stop=False)
            nc.tensor.matmul(p2T_ps[:], w2t1[:], hT1[:], start=False, stop=True)
            p2T = cpool.tile([P, L], f32, tag="p2T")
            nc.vector.tensor_copy(p2T[:], p2T_ps[:])

            # ---- PE: p1bT[p, i] = W1 @ hrT + bias ⊗ ones  ----
            p1b_ps = ppool.tile([P, P], f32, tag="p1ps")
            nc.tensor.matmul(p1b_ps[:], w1t0[:], hrT0[:], start=True, stop=False)
            nc.tensor.matmul(p1b_ps[:], w1t1[:], hrT1[:], start=False, stop=False)
            # bias term: lhsT = bias (K=1, M=128 -> partition p), rhs = ones
            nc.tensor.matmul(p1b_ps[:], bias_sb[:], ones_sb[:], start=False, stop=True)
            p1bT = cpool.tile([P, P], f32, tag="p1bT")
            nc.vector.tensor_copy(p1bT[:], p1b_ps[:])

            # ---- main loop: 128 broadcast-adds + 16 stores of 2 MB ----
            for g in range(n_groups):
                ot = opool.tile([P, G * L], f32, tag="ot")
                for u in range(G):
                    i = g * G + u
                    dst = ot[:, u * L : (u + 1) * L]
                    if i < act_rows:
                        # ACT engine: out = Identity(in * 1.0 + bias_per_part)
                        nc.scalar.add(dst, p2T[:], p1bT[:, i : i + 1])
                    else:
                        nc.vector.tensor_scalar_add(dst, p2T[:], p1bT[:, i : i + 1])
                # Alternate the two HWDGE rings (SP via nc.sync, ACT via
                # nc.scalar) so consecutive stores overlap instead of
                # serializing on one FIFO.
                dma_eng = nc.sync if g % 2 == 0 else nc.scalar
                dma_eng.dma_start(
                    out=out_d[:, g * G * L : (g + 1) * G * L], in_=ot[:]
                )

    nc.finalize()  # Bacc: regalloc + codegen passes; required before compile
    return nc


_PROGRAM_CACHE = {}


def _get_program(act_rows=0):
    key = act_rows
    if key not in _PROGRAM_CACHE:
        _PROGRAM_CACHE[key] = _build_program(act_rows)
    return _PROGRAM_CACHE[key]


def _make_in_maps(h, W, b):
    h = np.ascontiguousarray(np.asarray(h, dtype=np.float32))
    W = np.ascontiguousarray(np.asarray(W, dtype=np.float32))
    b = np.ascontiguousarray(np.asarray(b, dtype=np.float32))
    assert h.shape == (B, L, D) and W.shape == (P, 2 * D) and b.shape == (P,)

    hT = np.ascontiguousarray(h.transpose(0, 2, 1))  # (B, D, L)
    WT = np.ascontiguousarray(W.T)  # (2D, P)
    bias = np.ascontiguousarray(b.reshape(1, P))

    in_maps = []
    for c in range(NCORES):
        bb = c // BLOCKS_PER_B
        i0 = (c % BLOCKS_PER_B) * P
        in_maps.append(
            {
                "hT": hT[bb],
                "hrT": np.ascontiguousarray(hT[bb][:, i0 : i0 + P]),
                "WT": WT,
                "bias": bias,
            }
        )
    return in_maps


def _gather(results):
    # per-core shard: (P, P*L) = (p, i_local*L + j) -> full (B, L, L, P)
    shards = np.stack([np.asarray(r["out"]) for r in results])  # (8, P, P*L)
    shards = shards.reshape(NCORES, P, P, L)  # (core, p, i_local, j)
    out = shards.transpose(0, 2, 3, 1)  # (core, i_local, j, p)
    return np.ascontiguousarray(out).reshape(B, L, L, P)


def run_on_device(h, W, b, *, trace=False, tmpdir=None, act_rows=0):
    """Run the kernel; returns (output, BassKernelResults)."""
    from concourse.bass_utils import run_bass_kernel_spmd

    nc = _get_program(act_rows)
    in_maps = _make_in_maps(h, W, b)
    res = run_bass_kernel_spmd(
        nc, in_maps, list(range(NCORES)), trace=trace, tmpdir=tmpdir
    )
    return _gather(res.results), res


def kernel(h, W, b):
    out, _ = run_on_device(h, W, b)
    return out


# revision 12
# speedup vs baseline: 1.1668x; 1.1668x over previous
"""Pairwise module kernel for Trainium2 (8 NeuronCores, SPMD).

Computes out[b, i, j, p] = (h[b,i,:] @ W1.T)[p] + (h[b,j,:] @ W2.T)[p] + bias[p]
where W1 = W[:, :D], W2 = W[:, D:].

Shapes (hardcoded): h (2, 512, 256) f32, W (128, 512) f32, b (128,) f32.
Output: (2, 512, 512, 128) f32 = 256 MB.

Sharding: the (B*L = 1024) "query rows" i are split into 8 contiguous blocks
of 128; core c handles batch b = c // 4, rows i in [(c%4)*128, (c%4)*128+128).

Per-core device program:
  - load hT (d-major h[b].T), hrT (this core's 128 rows, d-major), WT, bias
  - PE: p2T[p, j]  = W2 @ h[b].T          (2 accumulating matmuls, K=128 each)
        p1bT[p, i] = W1 @ h_rows.T + bias (2 matmuls + rank-1 bias matmul)
  - DVE: for each of the 128 rows i: one tensor_scalar add
        out_tile[p, j] = p2T[p, j] + p1bT[p, i]   (per-partition scalar bcast)
  - DMA: store (p, i, j)-layout shard to DRAM in 2 MB contiguous chunks.

The per-core DRAM shard is (p, i_local, j); the host transposes/reassembles
into the (b, i, j, p) full output (host-side layout work only, no arithmetic).
"""

import numpy as np

B, L, D, P = 2, 512, 256, 128
NCORES = 8
BLOCKS_PER_B = 4  # 4 i-blocks of 128 per batch element
G = 8  # i-rows per output SBUF tile / per store DMA (G*512*128*4 = 2 MB)
F32 = None  # set in _build (mybir import kept inside functions)


def _build_program(act_rows=0):
    """Build the SPMD Bass program (identical on all 8 cores).

    act_rows: how many of the 128 broadcast-add rows to route to the scalar
    (ACT) engine instead of the vector engine (DVE) — load-balancing knob.
    """
    import concourse.bacc as bacc
    import concourse.mybir as mybir
    from concourse.tile import TileContext

    f32 = mybir.dt.float32
    nc = bacc.Bacc("TRN2", target_bir_lowering=False, debug=False, enable_partition_id=False)

    hT_d = nc.dram_tensor("hT", [D, L], f32, kind="ExternalInput")  # h[b].T
    hrT_d = nc.dram_tensor("hrT", [D, P], f32, kind="ExternalInput")  # rows.T
    WT_d = nc.dram_tensor("WT", [2 * D, P], f32, kind="ExternalInput")  # W.T
    bias_d = nc.dram_tensor("bias", [1, P], f32, kind="ExternalInput")
    out_d = nc.dram_tensor("out", [P, P * L], f32, kind="ExternalOutput")

    n_groups = P // G

    with TileContext(nc) as tc:
        with (
            tc.tile_pool(name="const", bufs=1) as cpool,
            tc.tile_pool(name="outp", bufs=4) as opool,
            tc.tile_pool(name="psum", bufs=2, space="PSUM") as ppool,
        ):
            # ---- load inputs (all tiny: ~1 MB total) ----
            # p2T's deps (hT, w2t) go first, split across both HWDGE rings,
            # so the PE -> DVE -> store pipeline lights up as early as
            # possible; p1bT's deps follow.
            hT0 = cpool.tile([P, L], f32, tag="hT0")
            hT1 = cpool.tile([P, L], f32, tag="hT1")
            w1t0 = cpool.tile([P, P], f32, tag="w1t0")
            w1t1 = cpool.tile([P, P], f32, tag="w1t1")
            w2t0 = cpool.tile([P, P], f32, tag="w2t0")
            w2t1 = cpool.tile([P, P], f32, tag="w2t1")
            hrT0 = cpool.tile([P, P], f32, tag="hrT0")
            hrT1 = cpool.tile([P, P], f32, tag="hrT1")
            bias_sb = cpool.tile([1, P], f32, tag="bias")

            nc.sync.dma_start(out=hT0[:], in_=hT_d[0:P, :])
            nc.scalar.dma_start(out=w2t0[:], in_=WT_d[2 * P : 3 * P, :])
            nc.scalar.dma_start(out=w2t1[:], in_=WT_d[3 * P : 4 * P, :])
            nc.sync.dma_start(out=hT1[:], in_=hT_d[P : 2 * P, :])
            nc.scalar.dma_start(out=hrT0[:], in_=hrT_d[0:P, :])
            nc.sync.dma_start(out=w1t0[:], in_=WT_d[0:P, :])
            nc.scalar.dma_start(out=hrT1[:], in_=hrT_d[P : 2 * P, :])
            nc.sync.dma_start(out=w1t1[:], in_=WT_d[P : 2 * P, :])
            nc.scalar.dma_start(out=bias_sb[:], in_=bias_d[:, :])

            ones_sb = cpool.tile([1, P], f32, tag="ones")
            nc.vector.memset(ones_sb[:], 1.0)

            # ---- PE: p2T[p, j] = W2 @ hT  (out (128, 512), K = 256 in 2) ----
            p2T_ps = ppool.tile([P, L], f32, tag="p2ps")
            nc.tensor.matmul(p2T_ps[:], w2t0[:], hT0[:], start=True, stop=False)
            nc.tensor.matmul(p2T_ps[:], w2t1[:], hT1[:], start=False, stop=True)
            p2T = cpool.tile([P, L], f32, tag="p2T")
            nc.vector.tensor_copy(p2T[:], p2T_ps[:])

            # ---- PE: p1bT[p, i] = W1 @ hrT + bias ⊗ ones  ----
            p1b_ps = ppool.tile([P, P], f32, tag="p1ps")
            nc.tensor.matmul(p1b_ps[:], w1t0[:], hrT0[:], start=True, stop=False)
            nc.tensor.matmul(p1b_ps[:], w1t1[:], hrT1[:], start=False, stop=False)
            # bias term: lhsT = bias (K=1, M=128 -> partition p), rhs = ones
            nc.tensor.matmul(p1b_ps[:], bias_sb[:], ones_sb[:], start=False, stop=True)
            p1bT = cpool.tile([P, P], f32, tag="p1bT")
            nc.vector.tensor_copy(p1bT[:], p1b_ps[:])

            # ---- main loop: 128 broadcast-adds + 16 stores of 2 MB ----
            for g in range(n_groups):
                ot = opool.tile([P, G * L], f32, tag="ot")
                for u in range(G):
                    i = g * G + u
                    dst = ot[:, u * L : (u + 1) * L]
                    if i < act_rows:
                        # ACT engine: out = Identity(in * 1.0 + bias_per_part)
                        nc.scalar.add(dst, p2T[:], p1bT[:, i : i + 1])
                    else:
                        nc.vector.tensor_scalar_add(dst, p2T[:], p1bT[:, i : i + 1])
                # Store each tile as two half-stores issued on the two HWDGE
                # rings (SP via nc.sync, ACT via nc.scalar) concurrently, so
                # neither ring serializes whole tiles behind the other.
                half = G * L // 2
                base = g * G * L
                nc.sync.dma_start(
                    out=out_d[:, base : base + half], in_=ot[:, :half]
                )
                nc.scalar.dma_start(
                    out=out_d[:, base + half : base + G * L], in_=ot[:, half:]
                )

    nc.finalize()  # Bacc: regalloc + codegen passes; required before compile
    return nc


_PROGRAM_CACHE = {}


def _get_program(act_rows=0):
    key = act_rows
    if key not in _PROGRAM_CACHE:
        _PROGRAM_CACHE[key] = _build_program(act_rows)
    return _PROGRAM_CACHE[key]


def _make_in_maps(h, W, b):
    h = np.ascontiguousarray(np.asarray(h, dtype=np.float32))
    W = np.ascontiguousarray(np.asarray(W, dtype=np.float32))
    b = np.ascontiguousarray(np.asarray(b, dtype=np.float32))
    assert h.shape == (B, L, D) and W.shape == (P, 2 * D) and b.shape == (P,)

    hT = np.ascontiguousarray(h.transpose(0, 2, 1))  # (B, D, L)
    WT = np.ascontiguousarray(W.T)  # (2D, P)
    bias = np.ascontiguousarray(b.reshape(1, P))

    in_maps = []
    for c in range(NCORES):
        bb = c // BLOCKS_PER_B
        i0 = (c % BLOCKS_PER_B) * P
        in_maps.append(
            {
                "hT": hT[bb],
                "hrT": np.ascontiguousarray(hT[bb][:, i0 : i0 + P]),
                "WT": WT,
                "bias": bias,
            }
        )
    return in_maps


def _gather(results):
    # per-core shard: (P, P*L) = (p, i_local*L + j) -> full (B, L, L, P)
    shards = np.stack([np.asarray(r["out"]) for r in results])  # (8, P, P*L)
    shards = shards.reshape(NCORES, P, P, L)  # (core, p, i_local, j)
    out = shards.transpose(0, 2, 3, 1)  # (core, i_local, j, p)
    return np.ascontiguousarray(out).reshape(B, L, L, P)


def run_on_device(h, W, b, *, trace=False, tmpdir=None, act_rows=0):
    """Run the kernel; returns (output, BassKernelResults)."""
    from concourse.bass_utils import run_bass_kernel_spmd

    nc = _get_program(act_rows)
    in_maps = _make_in_maps(h, W, b)
    res = run_bass_kernel_spmd(
        nc, in_maps, list(range(NCORES)), trace=trace, tmpdir=tmpdir
    )
    return _gather(res.results), res


def kernel(h, W, b):
    out, _ = run_on_device(h, W, b)
    return out
